# revision 1
# baseline (speedup 1.0000x reference)
"""Density-weighted Manhattan FPS sampler on 8 TRN2 NeuronCores.

Strategy: data-parallel over batch. Each core runs one batch end-to-end
(cores 4-7 duplicate batches 0-3). Two phases per core:

1. Density: pairwise squared-euclidean counts within radius R.
   i-points along 128 partitions (per-partition bias scalars), j-points
   replicated along the free dim; ACT does fused (xj - xi)^2 via
   Square(scale*in + bias); DVE sums components and counts d2 <= R^2 with
   a fused is_le + add-accumulate. Bit-exact vs the XLA reference:
   (dx^2 + dy^2) + dz^2, compare <= f32(0.16000000000000003).

2. FPS loop (2048 sequential steps) on a single 32-partition quadrant:
   d = (|dx| + |dy|) + |2z - 2pz| via ACT Abs + DVE; min-distance update;
   key = md * (1/density); argmax via per-row reduce_max + a 32x32
   stream-transpose fold; winner coords/index extracted with fused
   a fused scalar_tensor_tensor (is_ge -> mult -> sum-accum) op and a
   second transpose fold; the winner's coordinates are fetched with a
   register-indexed dynamic slice (reg_load + DynSlice) from an
   interleaved flat copy of the points, then broadcast via 32x32
   transposes. All f32 ops are IEEE-exact so the trajectory matches the
   reference bit-for-bit (required: min argmax margin on this input is
   ~7e-8, about one ulp).

   Measured on TRN2: ~7.3 us per FPS iteration, ~16 ms total device time
   (density ~1.6 ms). The loop is a single serial dependency chain
   (argmax_t -> coords -> distances_{t+1}), so extra cores cannot shorten
   it; cores 4-7 duplicate work and their outputs are ignored.
"""
import numpy as np

import concourse.bacc as bacc
import concourse.bass as bass
import concourse.mybir as mybir
import concourse.tile as tile
from concourse.bass_utils import run_bass_kernel_spmd

F32 = mybir.dt.float32
I32 = mybir.dt.int32
Alu = mybir.AluOpType
Act = mybir.ActivationFunctionType

B = 4
N = 8192
NPOINT = 2048
R2 = float(np.float32(0.16000000000000003))  # f32(0.4*0.4 in f64), bits 0x3e23d70a
MD_INIT = 1e10

LAST_PERF = None


def build_nc(n=N, npoint=NPOINT, ct=2048, fps_unroll=8, loop_mode="for_i",
             ablate=(), bench_repeats=1):
    """Build the SPMD Bass module. n must be divisible by 256 and ct;
    npoint divisible by fps_unroll. `ablate` (timing experiments only —
    breaks correctness): subset of {"act","dist","fold1","stt","fold2"}."""
    fp, ff = 32, n // 32          # FPS layout [32, ff]
    dp, df = 128, n // 128        # density i-layout [128, df]
    nct = n // ct                 # column tiles per row tile

    nc = bacc.Bacc("TRN2", target_bir_lowering=False, debug=True)

    # --- inputs (host-prepared layouts) ---
    xf_d = nc.dram_tensor("xf", [fp, ff], F32, kind="ExternalInput")
    yf_d = nc.dram_tensor("yf", [fp, ff], F32, kind="ExternalInput")
    zf_d = nc.dram_tensor("zf", [fp, ff], F32, kind="ExternalInput")
    iota_d = nc.dram_tensor("iota", [fp, ff], F32, kind="ExternalInput")
    xi_d = nc.dram_tensor("xi", [dp, df], F32, kind="ExternalInput")
    yi_d = nc.dram_tensor("yi", [dp, df], F32, kind="ExternalInput")
    zi_d = nc.dram_tensor("zi", [dp, df], F32, kind="ExternalInput")
    xj_d = nc.dram_tensor("xj", [1, n], F32, kind="ExternalInput")
    yj_d = nc.dram_tensor("yj", [1, n], F32, kind="ExternalInput")
    zj_d = nc.dram_tensor("zj", [1, n], F32, kind="ExternalInput")
    pflat_d = nc.dram_tensor("pflat", [1, 3 * n], F32, kind="ExternalInput")
    ntrip_d = nc.dram_tensor("ntrip", [1, 1], I32, kind="ExternalInput")

    # --- outputs ---
    idx_out = nc.dram_tensor("idx_out", [npoint], I32, kind="ExternalOutput")
    dens_out = nc.dram_tensor("dens_out", [n], F32, kind="ExternalOutput")
    md_out = nc.dram_tensor("md_out", [fp, ff], F32, kind="ExternalOutput")

    dens_dram = nc.dram_tensor("dens_dram", [n], F32)

    with tile.TileContext(nc) as tc:
        if True:
            # ---------------- density phase ----------------
            with tc.tile_pool(name="dens", bufs=1) as dpp:
                xi_t = dpp.tile([dp, df], F32)
                yi_t = dpp.tile([dp, df], F32)
                zi_t = dpp.tile([dp, df], F32)
                nc.sync.dma_start(xi_t[:], xi_d[:])
                nc.sync.dma_start(yi_t[:], yi_d[:])
                nc.sync.dma_start(zi_t[:], zi_d[:])
                nxi_t = dpp.tile([dp, df], F32)
                nyi_t = dpp.tile([dp, df], F32)
                nzi_t = dpp.tile([dp, df], F32)
                nc.vector.tensor_scalar(nxi_t[:], xi_t[:], -1.0, None, Alu.mult)
                nc.vector.tensor_scalar(nyi_t[:], yi_t[:], -1.0, None, Alu.mult)
                nc.vector.tensor_scalar(nzi_t[:], zi_t[:], -1.0, None, Alu.mult)

                xj_t = dpp.tile([dp, n], F32)
                yj_t = dpp.tile([dp, n], F32)
                zj_t = dpp.tile([dp, n], F32)
                nc.sync.dma_start(xj_t[:], xj_d[:].broadcast_to((dp, n)))
                nc.sync.dma_start(yj_t[:], yj_d[:].broadcast_to((dp, n)))
                nc.sync.dma_start(zj_t[:], zj_d[:].broadcast_to((dp, n)))

                pcnt = dpp.tile([dp, df * nct], F32)

                with tc.tile_pool(name="dscratch", bufs=2) as sp:
                    for rt in range(df):
                        for c in range(nct):
                            cs = slice(c * ct, (c + 1) * ct)
                            sqx = sp.tile([dp, ct], F32, tag="sqx")
                            sqy = sp.tile([dp, ct], F32, tag="sqy")
                            sqz = sp.tile([dp, ct], F32, tag="sqz")
                            nc.scalar.activation(sqx[:], xj_t[:, cs], Act.Square,
                                                 bias=nxi_t[:, rt:rt + 1], scale=1.0)
                            nc.scalar.activation(sqy[:], yj_t[:, cs], Act.Square,
                                                 bias=nyi_t[:, rt:rt + 1], scale=1.0)
                            nc.scalar.activation(sqz[:], zj_t[:, cs], Act.Square,
                                                 bias=nzi_t[:, rt:rt + 1], scale=1.0)
                            # in-place: sqx <- (sqx+sqy) ; sqx <- sqx+sqz ;
                            # sqy <- (sqx <= R2) with count accum
                            nc.vector.tensor_tensor(sqx[:], sqx[:], sqy[:], Alu.add)
                            nc.vector.tensor_tensor(sqx[:], sqx[:], sqz[:], Alu.add)
                            nc.vector.tensor_scalar(
                                sqy[:], sqx[:], R2, None, Alu.is_le, Alu.add,
                                accum_out=pcnt[:, rt * nct + c: rt * nct + c + 1])

                dens_t = dpp.tile([dp, df], F32)
                if nct > 1:
                    nc.vector.reduce_sum(
                        dens_t[:],
                        pcnt[:].rearrange("p (a b) -> p a b", a=df),
                        axis=mybir.AxisListType.X)
                else:
                    nc.vector.tensor_copy(dens_t[:], pcnt[:])

                # relayout [128, df] (j = rt*128 + p) -> linear dram
                dd2 = dens_dram[:].rearrange("(a b) -> a b", a=df)  # [df, 128]
                nc.sync.dma_start(dd2.transpose([1, 0]), dens_t[:])
                nc.sync.dma_start(dens_out[:], dens_dram[:])

        with tc.tile_pool(name="fps", bufs=1) as pp:
            xft = pp.tile([fp, ff], F32)
            yft = pp.tile([fp, ff], F32)
            z2ft = pp.tile([fp, ff], F32)
            iot = pp.tile([fp, ff], F32)
            mdt = pp.tile([fp, ff], F32)
            pent = pp.tile([fp, ff], F32)
            penf = pp.tile([fp, ff], F32)   # raw density in fps layout
            npq = pp.tile([fp, 3], F32)     # negated px, py, pz2
            trace = pp.tile([fp, npoint + fps_unroll + 66], F32)

            nc.sync.dma_start(xft[:], xf_d[:])
            nc.sync.dma_start(yft[:], yf_d[:])
            nc.sync.dma_start(iot[:], iota_d[:])
            zf_tmp = pp.tile([fp, ff], F32)
            nc.sync.dma_start(zf_tmp[:], zf_d[:])
            nc.vector.tensor_scalar(z2ft[:], zf_tmp[:], 2.0, None, Alu.mult)

            # load density in fps layout + reciprocal
            nc.sync.dma_start(penf[:], dens_dram[:].rearrange("(a b) -> a b", a=fp))
            nc.vector.reciprocal(pent[:], penf[:])

            # ---------------- FPS init ----------------
            nc.vector.memset(mdt[:], MD_INIT)
            nc.vector.memset(trace[:], 0.0)

            e1 = pp.tile([fp, 32], F32)
            e1t = pp.tile([fp, 32], F32)
            nc.vector.memset(e1[:], 0.0)

            # ---------------- FPS loop ----------------
            ax = pp.tile([fp, ff], F32)
            ay = pp.tile([fp, ff], F32)
            az_a = pp.tile([fp, ff], F32)
            az_b = pp.tile([fp, ff], F32)
            s12 = pp.tile([fp, ff], F32)
            dd = pp.tile([fp, ff], F32)
            key = pp.tile([fp, ff], F32)
            rowmax = pp.tile([fp, 1], F32)
            mglob = pp.tile([fp, 1], F32)
            junk = pp.tile([fp, ff], F32)
            e2 = pp.tile([fp, 32], F32)     # col 0 = per-row sum(mask*iota)
            e2t = pp.tile([fp, 32], F32)
            jgall = pp.tile([fp, 1], F32)   # partition 0 = selected index
            jgi = pp.tile([1, 1], I32)
            g0 = pp.tile([fp, 32], F32)     # col 0 = px replicated
            g1 = pp.tile([fp, 32], F32)     # col 0 = py replicated
            g2 = pp.tile([fp, 32], F32)     # col 0 = pz2 replicated
            flat3 = pp.tile([1, 3 * n], F32)  # interleaved x,y,2z on part. 0
            gbuf = pp.tile([1, 4], F32)       # gathered (px, py, pz2)
            jg3 = pp.tile([1, 1], F32)        # 3 * selected index
            nc.vector.memset(e2[:], 0.0)
            nc.sync.dma_start(flat3[:], pflat_d[:])
            z2v = flat3[0:1, :].rearrange("a (b c) -> a b c", c=3)[:, :, 2:3]
            nc.vector.tensor_scalar(z2v, z2v, 2.0, None, Alu.mult)

            _ = ntrip_d  # reserved: runtime trip count (unused)

            # dynamic-index register (DVE) for the winner-coordinate gather
            jreg = nc.alloc_register(mybir.EngineType.DVE, "jreg")
            jsv = bass.make_scalar_value(
                bass.RegisterHandles([jreg]), min_val=0, max_val=3 * (n - 1))

            def gather_coords():
                """Broadcast flat3[3j + k] to gk[:, 0] for k = 0, 1, 2."""
                for k, gk in enumerate((g0, g1, g2)):
                    src = flat3[0:1, k:][:, bass.DynSlice(jsv, 1)]
                    nc.vector.tensor_copy(e1[0:1, :], src.broadcast_to((1, 32)))
                    nc.vector.transpose(gk[:], e1[:])
                # negated biases for the ACT Abs ops
                nc.vector.tensor_scalar(npq[:, 0:1], g0[:, 0:1], -1.0, None,
                                        Alu.mult)
                nc.vector.tensor_scalar(npq[:, 1:2], g1[:, 0:1], -1.0, None,
                                        Alu.mult)
                nc.vector.tensor_scalar(npq[:, 2:3], g2[:, 0:1], -1.0, None,
                                        Alu.mult)

            # seed with point 0
            nc.vector.reg_mov(jreg, 0)
            gather_coords()

            def body(iv):
                # distance update: ACT computes |x-px|, |y-py|; DVE |2z-2pz|
                if "alldve" in ablate:
                    nc.vector.tensor_scalar(ax[:], xft[:], g0[:, 0:1], None,
                                            Alu.subtract)
                    nc.vector.tensor_scalar(s12[:], xft[:], -1.0,
                                            g0[:, 0:1], Alu.mult, Alu.add)
                    nc.vector.tensor_tensor(ax[:], ax[:], s12[:], Alu.max)
                    nc.vector.tensor_scalar(ay[:], yft[:], g1[:, 0:1], None,
                                            Alu.subtract)
                    nc.vector.tensor_scalar(s12[:], yft[:], -1.0,
                                            g1[:, 0:1], Alu.mult, Alu.add)
                    nc.vector.tensor_tensor(ay[:], ay[:], s12[:], Alu.max)
                elif "act" not in ablate:
                    nc.scalar.activation(ax[:], xft[:], Act.Abs,
                                         bias=npq[:, 0:1], scale=1.0)
                    nc.scalar.activation(ay[:], yft[:], Act.Abs,
                                         bias=npq[:, 1:2], scale=1.0)
                if "dist" not in ablate:
                    if "dvez" not in ablate:
                        nc.scalar.activation(az_a[:], z2ft[:], Act.Abs,
                                             bias=npq[:, 2:3], scale=1.0)
                    else:
                        # az = |2z - 2pz| = max(2z-2pz, -(2z-2pz)) on DVE
                        nc.vector.tensor_scalar(az_a[:], z2ft[:], g2[:, 0:1],
                                                None, Alu.subtract)
                        nc.vector.tensor_scalar(az_b[:], z2ft[:], -1.0,
                                                g2[:, 0:1], Alu.mult, Alu.add)
                        nc.vector.tensor_tensor(az_a[:], az_a[:], az_b[:],
                                                Alu.max)
                    # d = (ax + ay) + az  (reference order); the first add
                    # runs on GPSIMD, overlapped with ACT's az and DVE's tail
                    if "gps12" in ablate:
                        nc.gpsimd.tensor_tensor(s12[:], ax[:], ay[:], Alu.add)
                    else:
                        nc.vector.tensor_tensor(s12[:], ax[:], ay[:], Alu.add)
                    nc.vector.tensor_tensor(dd[:], s12[:], az_a[:], Alu.add)
                    nc.vector.tensor_tensor(mdt[:], mdt[:], dd[:], Alu.min)
                    nc.vector.tensor_tensor(key[:], mdt[:], pent[:], Alu.mult)
                # global max fold
                if "fold1" not in ablate:
                    nc.vector.reduce_max(rowmax[:], key[:],
                                         axis=mybir.AxisListType.X)
                    nc.vector.tensor_copy(e1[:], rowmax[:].broadcast_to((fp, 32)))
                    nc.vector.transpose(e1t[:], e1[:])
                    nc.vector.reduce_max(mglob[:], e1t[:],
                                         axis=mybir.AxisListType.X)
                # winner index: per-row sum((key >= m) * iota) -> e2 col 0,
                # fold to partition 0, load into jreg
                if "stt" not in ablate:
                    nc.vector.scalar_tensor_tensor(
                        junk[:], key[:], mglob[:], iot[:],
                        op0=Alu.is_ge, op1=Alu.mult,
                        accum_out=e2[:, 0:1])
                if "fold2" not in ablate:
                    nc.vector.transpose(e2t[:], e2[:])
                    nc.vector.reduce_sum(jgall[:], e2t[:],
                                         axis=mybir.AxisListType.X)
                    nc.vector.tensor_scalar(jgi[:], jgall[0:1, 0:1], 3.0,
                                            None, Alu.mult)
                    nc.vector.reg_load(jreg, jgi[0:1, 0:1])
                    gather_coords()
                # record selected index (output position iv+1)
                nc.vector.tensor_copy(
                    trace[0:1, 1:][:, bass.DynSlice(iv, 1)], jgall[0:1, 0:1])

            if loop_mode == "unrolled":
                for t in range(npoint):
                    body(t)
            else:
                hints = (mybir.EngineType.DVE, mybir.EngineType.Activation) \
                    if "hint" in ablate else ()
                stag = "stag" in ablate
                for _rep in range(bench_repeats):
                    with tc.For_i(0, npoint, fps_unroll,
                                  hint_engines=hints,
                                  staggered_reset=stag) as iv:
                        for k in range(fps_unroll):
                            body(iv + k)

            # ---------------- outputs ----------------
            idx32 = pp.tile([1, npoint], I32)
            nc.vector.tensor_copy(idx32[:], trace[0:1, 0:npoint])
            nc.sync.dma_start(idx_out[:].rearrange("(a b) -> a b", a=1), idx32[:])
            nc.sync.dma_start(md_out[:], mdt[:])

    nc.finalize()
    return nc


def make_in_maps(points, n=N, n_cores=8, ntrip=NPOINT):
    """Per-core host-side input layouts. Core c handles batch c % B."""
    fp, ff = 32, n // 32
    dp, df = 128, n // 128
    iota = np.arange(n, dtype=np.float32).reshape(fp, ff)
    in_maps = []
    for c in range(n_cores):
        b = c % points.shape[0]
        p = np.ascontiguousarray(points[b])  # [n, 3] f32
        m = {
            "xf": p[:, 0].reshape(fp, ff).copy(),
            "yf": p[:, 1].reshape(fp, ff).copy(),
            "zf": p[:, 2].reshape(fp, ff).copy(),
            "iota": iota,
            "xi": np.ascontiguousarray(p[:, 0].reshape(df, dp).T),
            "yi": np.ascontiguousarray(p[:, 1].reshape(df, dp).T),
            "zi": np.ascontiguousarray(p[:, 2].reshape(df, dp).T),
            "xj": p[:, 0].reshape(1, n).copy(),
            "yj": p[:, 1].reshape(1, n).copy(),
            "zj": p[:, 2].reshape(1, n).copy(),
            "pflat": np.ascontiguousarray(p.reshape(1, 3 * n)),
            "ntrip": np.full((1, 1), ntrip, np.int32),
        }
        in_maps.append(m)
    return in_maps


_NC_CACHE = {}


def kernel(points, features=None, npoint=None, **_unused):
    """Full-input entry point: points [4, 8192, 3] f32 -> [4, 2048] int32."""
    global LAST_PERF
    points = np.asarray(points, dtype=np.float32)
    assert points.shape == (B, N, 3), points.shape
    npt = int(npoint) if npoint is not None else NPOINT
    assert npt == NPOINT, f"kernel hardcodes npoint={NPOINT}, got {npt}"

    if "nc" not in _NC_CACHE:
        _NC_CACHE["nc"] = build_nc()
    nc = _NC_CACHE["nc"]

    in_maps = make_in_maps(points)
    res = run_bass_kernel_spmd(nc, in_maps, core_ids=list(range(8)))
    LAST_PERF = res
    out = np.stack([res.results[b]["idx_out"] for b in range(B)], axis=0)
    return out.astype(np.int32)



# revision 2
# speedup vs baseline: 1.4127x; 1.4127x over previous
"""Density-weighted Manhattan FPS sampler on 8 TRN2 NeuronCores — v2.

Data-parallel over batch (cores 4-7 duplicate batches 0-3). Key design
points (all bit-exact vs the XLA reference trajectory):

- FPS state in a [128, 64] layout (full partition width); elementwise ops
  cost ~67 ns of DVE cycles + ~320 ns fixed dependency latency, so the
  loop is latency-bound and the design minimizes serial dependent hops.
- Key-state reformulation: key_j = min(key_j, round(d_j * pen_j)) equals
  the reference round(min_dist_j * pen_j) because multiplication by
  pen_j > 0 is monotone under round-to-nearest.
- |dx| and |dy| on ACT (Abs with scale=-1, bias=+coord), |2dz| as a
  max(u, -u) pair on DVE, summed in the reference order (ax+ay)+az.
- Per-partition winner candidates extracted with 4 pipelined
  scalar_tensor_tensor ops gated on the partition's own rowmax (they
  overlap the GPSIMD partition_all_reduce(max) that folds + broadcasts
  the global max), then one is_ge/mult gate pair zeroes non-winner
  partitions and a partition_all_reduce(add) broadcasts the winner's
  (index, px, py, 2pz) to every partition in one op.
- No register loads / dynamic-slice gathers anywhere in the loop.

Measured on TRN2 (npoint-delta): ~2.3-2.9 us steady-state per FPS step
(baseline v1: ~7.1 us); density phase unchanged (~1.6 ms).
"""
import numpy as np

import concourse.bacc as bacc
import concourse.bass as bass
import concourse.bass_isa as bass_isa
import concourse.mybir as mybir
import concourse.tile as tile
from concourse.bass_utils import run_bass_kernel_spmd

F32 = mybir.dt.float32
I32 = mybir.dt.int32
Alu = mybir.AluOpType
Act = mybir.ActivationFunctionType

B = 4
N = 8192
NPOINT = 2048
R2 = float(np.float32(0.16000000000000003))
MD_INIT = 1e10

LAST_PERF = None


def build_nc(n=N, npoint=NPOINT, ct=2048, fps_unroll=16, loop_mode="for_i",
             ay_act=True, ablate=(), bench_repeats=1):
    fp, ff = 128, n // 128     # FPS layout
    dp, df = 128, n // 128     # density i-layout
    nct = n // ct

    nc = bacc.Bacc("TRN2", target_bir_lowering=False, debug=True)

    # --- inputs (host-prepared layouts) ---
    xf_d = nc.dram_tensor("xf", [fp, ff], F32, kind="ExternalInput")
    yf_d = nc.dram_tensor("yf", [fp, ff], F32, kind="ExternalInput")
    zf_d = nc.dram_tensor("zf", [fp, ff], F32, kind="ExternalInput")
    iota_d = nc.dram_tensor("iota", [fp, ff], F32, kind="ExternalInput")
    xi_d = nc.dram_tensor("xi", [dp, df], F32, kind="ExternalInput")
    yi_d = nc.dram_tensor("yi", [dp, df], F32, kind="ExternalInput")
    zi_d = nc.dram_tensor("zi", [dp, df], F32, kind="ExternalInput")
    xj_d = nc.dram_tensor("xj", [1, n], F32, kind="ExternalInput")
    yj_d = nc.dram_tensor("yj", [1, n], F32, kind="ExternalInput")
    zj_d = nc.dram_tensor("zj", [1, n], F32, kind="ExternalInput")
    seed_d = nc.dram_tensor("seed", [1, 4], F32, kind="ExternalInput")

    # --- outputs ---
    idx_out = nc.dram_tensor("idx_out", [npoint], I32, kind="ExternalOutput")
    dens_out = nc.dram_tensor("dens_out", [n], F32, kind="ExternalOutput")

    dens_dram = nc.dram_tensor("dens_dram", [n], F32)

    with tile.TileContext(nc) as tc:
        # ---------------- density phase (unchanged from v1) ----------------
        with tc.tile_pool(name="dens", bufs=1) as dpp:
            xi_t = dpp.tile([dp, df], F32)
            yi_t = dpp.tile([dp, df], F32)
            zi_t = dpp.tile([dp, df], F32)
            nc.sync.dma_start(xi_t[:], xi_d[:])
            nc.sync.dma_start(yi_t[:], yi_d[:])
            nc.sync.dma_start(zi_t[:], zi_d[:])
            nxi_t = dpp.tile([dp, df], F32)
            nyi_t = dpp.tile([dp, df], F32)
            nzi_t = dpp.tile([dp, df], F32)
            nc.vector.tensor_scalar(nxi_t[:], xi_t[:], -1.0, None, Alu.mult)
            nc.vector.tensor_scalar(nyi_t[:], yi_t[:], -1.0, None, Alu.mult)
            nc.vector.tensor_scalar(nzi_t[:], zi_t[:], -1.0, None, Alu.mult)

            xj_t = dpp.tile([dp, n], F32)
            yj_t = dpp.tile([dp, n], F32)
            zj_t = dpp.tile([dp, n], F32)
            nc.sync.dma_start(xj_t[:], xj_d[:].broadcast_to((dp, n)))
            nc.sync.dma_start(yj_t[:], yj_d[:].broadcast_to((dp, n)))
            nc.sync.dma_start(zj_t[:], zj_d[:].broadcast_to((dp, n)))

            pcnt = dpp.tile([dp, df * nct], F32)

            with tc.tile_pool(name="dscratch", bufs=2) as sp:
                for rt in range(df):
                    for c in range(nct):
                        cs = slice(c * ct, (c + 1) * ct)
                        sqx = sp.tile([dp, ct], F32, tag="sqx")
                        sqy = sp.tile([dp, ct], F32, tag="sqy")
                        sqz = sp.tile([dp, ct], F32, tag="sqz")
                        nc.scalar.activation(sqx[:], xj_t[:, cs], Act.Square,
                                             bias=nxi_t[:, rt:rt + 1], scale=1.0)
                        nc.scalar.activation(sqy[:], yj_t[:, cs], Act.Square,
                                             bias=nyi_t[:, rt:rt + 1], scale=1.0)
                        nc.scalar.activation(sqz[:], zj_t[:, cs], Act.Square,
                                             bias=nzi_t[:, rt:rt + 1], scale=1.0)
                        nc.vector.tensor_tensor(sqx[:], sqx[:], sqy[:], Alu.add)
                        nc.vector.tensor_tensor(sqx[:], sqx[:], sqz[:], Alu.add)
                        nc.vector.tensor_scalar(
                            sqy[:], sqx[:], R2, None, Alu.is_le, Alu.add,
                            accum_out=pcnt[:, rt * nct + c: rt * nct + c + 1])

            dens_t = dpp.tile([dp, df], F32)
            if nct > 1:
                nc.vector.reduce_sum(
                    dens_t[:],
                    pcnt[:].rearrange("p (a b) -> p a b", a=df),
                    axis=mybir.AxisListType.X)
            else:
                nc.vector.tensor_copy(dens_t[:], pcnt[:])

            # relayout [128, df] (j = rt*128 + p) -> linear dram
            dd2 = dens_dram[:].rearrange("(a b) -> a b", a=df)  # [df, 128]
            nc.sync.dma_start(dd2.transpose([1, 0]), dens_t[:])
            nc.sync.dma_start(dens_out[:], dens_dram[:])

        with tc.tile_pool(name="fps", bufs=1) as pp:
            xft = pp.tile([fp, ff], F32)
            yft = pp.tile([fp, ff], F32)
            z2ft = pp.tile([fp, ff], F32)
            iot = pp.tile([fp, ff], F32)
            pent = pp.tile([fp, ff], F32)
            penf = pp.tile([fp, ff], F32)
            key = pp.tile([fp, ff], F32)
            trace = pp.tile([fp, npoint + fps_unroll + 66], F32)

            nc.sync.dma_start(xft[:], xf_d[:])
            nc.sync.dma_start(yft[:], yf_d[:])
            nc.sync.dma_start(iot[:], iota_d[:])
            zf_tmp = pp.tile([fp, ff], F32)
            nc.sync.dma_start(zf_tmp[:], zf_d[:])
            nc.vector.tensor_scalar(z2ft[:], zf_tmp[:], 2.0, None, Alu.mult)

            # density in fps layout (j = p*ff + c) + reciprocal
            nc.sync.dma_start(penf[:], dens_dram[:].rearrange("(a b) -> a b",
                                                              a=fp))
            nc.vector.reciprocal(pent[:], penf[:])

            # ---------------- FPS init ----------------
            # key-state: round(MD_INIT * pen); see module docstring.
            nc.vector.tensor_scalar(key[:], pent[:], MD_INIT, None, Alu.mult)
            nc.vector.memset(trace[:], 0.0)

            seed_t = pp.tile([1, 4], F32)
            nc.sync.dma_start(seed_t[:], seed_d[:])

            ax = pp.tile([fp, ff], F32)
            ay_a = pp.tile([fp, ff], F32)
            ay_b = pp.tile([fp, ff], F32)
            az_a = pp.tile([fp, ff], F32)
            az_b = pp.tile([fp, ff], F32)
            s12 = pp.tile([fp, ff], F32)
            dd = pp.tile([fp, ff], F32)
            dp_t = pp.tile([fp, ff], F32)
            junk4 = pp.tile([fp, 4, ff], F32)
            rowmax = pp.tile([fp, 1], F32)
            parmax = pp.tile([fp, 1], F32)
            gcs = pp.tile([fp, 4], F32)   # per-partition (idx, px, py, pz2)
            pm = pp.tile([fp, 1], F32)    # winner-partition mask
            gfin = pp.tile([fp, 4], F32)  # gated candidates
            gc = pp.tile([fp, 4], F32)    # globally reduced + broadcast

            # seed: gc = (0, x0, y0, 2*z0) on every partition
            nc.gpsimd.partition_broadcast(gc[:], seed_t[:])

            def body(iv):
                # |dx| on ACT: Abs(-x + px); |2dz| via max pair on DVE;
                # |dy| on ACT or DVE per `ay_act`
                if not ay_act:
                    nc.vector.tensor_scalar(ay_b[:], yft[:], -1.0,
                                            gc[:, 2:3], Alu.mult, Alu.add)
                    nc.vector.tensor_scalar(ay_a[:], yft[:], gc[:, 2:3],
                                            None, Alu.subtract)
                nc.vector.tensor_scalar(az_b[:], z2ft[:], -1.0, gc[:, 3:4],
                                        Alu.mult, Alu.add)
                nc.vector.tensor_scalar(az_a[:], z2ft[:], gc[:, 3:4], None,
                                        Alu.subtract)
                nc.scalar.activation(ax[:], xft[:], Act.Abs,
                                     bias=gc[:, 1:2], scale=-1.0)
                if ay_act:
                    nc.scalar.activation(ay_a[:], yft[:], Act.Abs,
                                         bias=gc[:, 2:3], scale=-1.0)
                else:
                    nc.vector.tensor_tensor(ay_a[:], ay_a[:], ay_b[:],
                                            Alu.max)
                nc.vector.tensor_tensor(az_a[:], az_a[:], az_b[:], Alu.max)
                nc.vector.tensor_tensor(s12[:], ax[:], ay_a[:], Alu.add)
                nc.vector.tensor_tensor(dd[:], s12[:], az_a[:], Alu.add)
                nc.vector.tensor_tensor(dp_t[:], dd[:], pent[:], Alu.mult)
                # key = min(key, dp); rowmax = max over free of new key
                nc.vector.tensor_tensor(key[:], key[:], dp_t[:], Alu.min)
                nc.vector.reduce_max(rowmax[:], key[:],
                                     axis=mybir.AxisListType.X)
                # global max (folded + broadcast) on GPSIMD, overlapped with
                # the per-partition candidate extraction below
                nc.gpsimd.partition_all_reduce(
                    parmax[:], rowmax[:], 128, bass_isa.ReduceOp.max)
                # per-partition candidate: (key >= rowmax_p) * v, sum of free
                # (separate junk output slices keep the 4 stts pipelined)
                nc.vector.scalar_tensor_tensor(
                    junk4[:, 0, :], key[:], rowmax[:, 0:1], iot[:],
                    op0=Alu.is_ge, op1=Alu.mult, accum_out=gcs[:, 0:1])
                nc.vector.scalar_tensor_tensor(
                    junk4[:, 1, :], key[:], rowmax[:, 0:1], xft[:],
                    op0=Alu.is_ge, op1=Alu.mult, accum_out=gcs[:, 1:2])
                nc.vector.scalar_tensor_tensor(
                    junk4[:, 2, :], key[:], rowmax[:, 0:1], yft[:],
                    op0=Alu.is_ge, op1=Alu.mult, accum_out=gcs[:, 2:3])
                nc.vector.scalar_tensor_tensor(
                    junk4[:, 3, :], key[:], rowmax[:, 0:1], z2ft[:],
                    op0=Alu.is_ge, op1=Alu.mult, accum_out=gcs[:, 3:4])
                # gate: winner partition keeps its candidate, others zero
                nc.vector.tensor_scalar(pm[:], rowmax[:], parmax[:, 0:1],
                                        None, Alu.is_ge)
                nc.vector.tensor_scalar(gfin[:], gcs[:], pm[:, 0:1], None,
                                        Alu.mult)
                nc.gpsimd.partition_all_reduce(
                    gc[:], gfin[:], 128, bass_isa.ReduceOp.add)
                # record selected index (output position iv+1)
                nc.vector.tensor_copy(
                    trace[0:1, 1:][:, bass.DynSlice(iv, 1)], gc[0:1, 0:1])

            if loop_mode == "unrolled":
                for t in range(npoint):
                    body(t)
            else:
                for _rep in range(bench_repeats):
                    with tc.For_i(0, npoint, fps_unroll) as iv:
                        for k in range(fps_unroll):
                            body(iv + k)

            # ---------------- outputs ----------------
            idx32 = pp.tile([1, npoint], I32)
            nc.vector.tensor_copy(idx32[:], trace[0:1, 0:npoint])
            nc.sync.dma_start(idx_out[:].rearrange("(a b) -> a b", a=1),
                              idx32[:])

    nc.finalize()
    return nc


def make_in_maps(points, n=N, n_cores=8, ntrip=NPOINT):
    """Per-core host-side input layouts. Core c handles batch c % B."""
    fp, ff = 128, n // 128
    dp, df = 128, n // 128
    iota = np.arange(n, dtype=np.float32).reshape(fp, ff)
    in_maps = []
    for c in range(n_cores):
        b = c % points.shape[0]
        p = np.ascontiguousarray(points[b])  # [n, 3] f32
        seed = np.array([[0.0, p[0, 0], p[0, 1],
                          np.float32(2.0) * p[0, 2]]], dtype=np.float32)
        m = {
            "xf": p[:, 0].reshape(fp, ff).copy(),
            "yf": p[:, 1].reshape(fp, ff).copy(),
            "zf": p[:, 2].reshape(fp, ff).copy(),
            "iota": iota,
            "xi": np.ascontiguousarray(p[:, 0].reshape(df, dp).T),
            "yi": np.ascontiguousarray(p[:, 1].reshape(df, dp).T),
            "zi": np.ascontiguousarray(p[:, 2].reshape(df, dp).T),
            "xj": p[:, 0].reshape(1, n).copy(),
            "yj": p[:, 1].reshape(1, n).copy(),
            "zj": p[:, 2].reshape(1, n).copy(),
            "seed": seed,
        }
        in_maps.append(m)
    return in_maps


_NC_CACHE = {}


def kernel(points, features=None, npoint=None, **_unused):
    """Full-input entry point: points [4, 8192, 3] f32 -> [4, 2048] int32."""
    global LAST_PERF
    points = np.asarray(points, dtype=np.float32)
    assert points.shape == (B, N, 3), points.shape
    npt = int(npoint) if npoint is not None else NPOINT
    assert npt == NPOINT, f"kernel hardcodes npoint={NPOINT}, got {npt}"

    if "nc" not in _NC_CACHE:
        _NC_CACHE["nc"] = build_nc()
    nc = _NC_CACHE["nc"]

    in_maps = make_in_maps(points)
    res = run_bass_kernel_spmd(nc, in_maps, core_ids=list(range(8)))
    LAST_PERF = res
    out = np.stack([res.results[b]["idx_out"] for b in range(B)], axis=0)
    return out.astype(np.int32)


# revision 4
# speedup vs baseline: 1.4827x; 1.0496x over previous
"""Density-weighted Manhattan FPS sampler on 8 TRN2 NeuronCores — v2.

Data-parallel over batch (cores 4-7 duplicate batches 0-3). Key design
points (all bit-exact vs the XLA reference trajectory):

- FPS state in a [128, 64] layout (full partition width); elementwise ops
  cost ~67 ns of DVE cycles + ~320 ns fixed dependency latency, so the
  loop is latency-bound and the design minimizes serial dependent hops.
- Key-state reformulation: key_j = min(key_j, round(d_j * pen_j)) equals
  the reference round(min_dist_j * pen_j) because multiplication by
  pen_j > 0 is monotone under round-to-nearest.
- |dx| and |dy| on ACT (Abs with scale=-1, bias=+coord), |2dz| as a
  max(u, -u) pair on DVE, summed in the reference order (ax+ay)+az.
- Per-partition winner candidates extracted with 4 pipelined
  scalar_tensor_tensor ops gated on the partition's own rowmax (they
  overlap the GPSIMD partition_all_reduce(max) that folds + broadcasts
  the global max), then one is_ge/mult gate pair zeroes non-winner
  partitions and a partition_all_reduce(add) broadcasts the winner's
  (index, px, py, 2pz) to every partition in one op.
- No register loads / dynamic-slice gathers anywhere in the loop.

Measured on TRN2 (npoint-delta): ~2.3-2.9 us steady-state per FPS step
(baseline v1: ~7.1 us); density phase unchanged (~1.6 ms).
"""
import numpy as np

import concourse.bacc as bacc
import concourse.bass as bass
import concourse.bass_isa as bass_isa
import concourse.mybir as mybir
import concourse.tile as tile
from concourse.bass_utils import run_bass_kernel_spmd

F32 = mybir.dt.float32
I32 = mybir.dt.int32
Alu = mybir.AluOpType
Act = mybir.ActivationFunctionType

B = 4
N = 8192
NPOINT = 2048
R2 = float(np.float32(0.16000000000000003))
MD_INIT = 1e10

LAST_PERF = None


def build_nc(n=N, npoint=NPOINT, ct=2048, fps_unroll=16, loop_mode="for_i",
             ay_act=True, ablate=(), bench_repeats=1):
    fp, ff = 128, n // 128     # FPS layout
    dp, df = 128, n // 128     # density i-layout
    nct = n // ct

    nc = bacc.Bacc("TRN2", target_bir_lowering=False, debug=True)

    # --- inputs (host-prepared layouts) ---
    xf_d = nc.dram_tensor("xf", [fp, ff], F32, kind="ExternalInput")
    yf_d = nc.dram_tensor("yf", [fp, ff], F32, kind="ExternalInput")
    zf_d = nc.dram_tensor("zf", [fp, ff], F32, kind="ExternalInput")
    iota_d = nc.dram_tensor("iota", [fp, ff], F32, kind="ExternalInput")
    xi_d = nc.dram_tensor("xi", [dp, df], F32, kind="ExternalInput")
    yi_d = nc.dram_tensor("yi", [dp, df], F32, kind="ExternalInput")
    zi_d = nc.dram_tensor("zi", [dp, df], F32, kind="ExternalInput")
    xj_d = nc.dram_tensor("xj", [1, n], F32, kind="ExternalInput")
    yj_d = nc.dram_tensor("yj", [1, n], F32, kind="ExternalInput")
    zj_d = nc.dram_tensor("zj", [1, n], F32, kind="ExternalInput")
    seed_d = nc.dram_tensor("seed", [1, 4], F32, kind="ExternalInput")

    # --- outputs ---
    idx_out = nc.dram_tensor("idx_out", [npoint], I32, kind="ExternalOutput")
    dens_out = nc.dram_tensor("dens_out", [n], F32, kind="ExternalOutput")

    dens_dram = nc.dram_tensor("dens_dram", [n], F32)

    with tile.TileContext(nc) as tc:
        # ---------------- density phase (unchanged from v1) ----------------
        with tc.tile_pool(name="dens", bufs=1) as dpp:
            xi_t = dpp.tile([dp, df], F32)
            yi_t = dpp.tile([dp, df], F32)
            zi_t = dpp.tile([dp, df], F32)
            nc.sync.dma_start(xi_t[:], xi_d[:])
            nc.sync.dma_start(yi_t[:], yi_d[:])
            nc.sync.dma_start(zi_t[:], zi_d[:])
            nxi_t = dpp.tile([dp, df], F32)
            nyi_t = dpp.tile([dp, df], F32)
            nzi_t = dpp.tile([dp, df], F32)
            nc.vector.tensor_scalar(nxi_t[:], xi_t[:], -1.0, None, Alu.mult)
            nc.vector.tensor_scalar(nyi_t[:], yi_t[:], -1.0, None, Alu.mult)
            nc.vector.tensor_scalar(nzi_t[:], zi_t[:], -1.0, None, Alu.mult)

            xj_t = dpp.tile([dp, n], F32)
            yj_t = dpp.tile([dp, n], F32)
            zj_t = dpp.tile([dp, n], F32)
            nc.sync.dma_start(xj_t[:], xj_d[:].broadcast_to((dp, n)))
            nc.sync.dma_start(yj_t[:], yj_d[:].broadcast_to((dp, n)))
            nc.sync.dma_start(zj_t[:], zj_d[:].broadcast_to((dp, n)))

            pcnt = dpp.tile([dp, df * nct], F32)

            with tc.tile_pool(name="dscratch", bufs=2) as sp:
                for rt in range(df):
                    for c in range(nct):
                        cs = slice(c * ct, (c + 1) * ct)
                        sqx = sp.tile([dp, ct], F32, tag="sqx")
                        sqy = sp.tile([dp, ct], F32, tag="sqy")
                        sqz = sp.tile([dp, ct], F32, tag="sqz")
                        nc.scalar.activation(sqx[:], xj_t[:, cs], Act.Square,
                                             bias=nxi_t[:, rt:rt + 1], scale=1.0)
                        nc.scalar.activation(sqy[:], yj_t[:, cs], Act.Square,
                                             bias=nyi_t[:, rt:rt + 1], scale=1.0)
                        nc.scalar.activation(sqz[:], zj_t[:, cs], Act.Square,
                                             bias=nzi_t[:, rt:rt + 1], scale=1.0)
                        # (sqx+sqy) on GPSIMD to offload DVE (exact f32 add);
                        # DVE does the final add and the compare+count
                        nc.gpsimd.tensor_tensor(sqx[:], sqx[:], sqy[:],
                                                Alu.add)
                        nc.vector.tensor_tensor(sqx[:], sqx[:], sqz[:], Alu.add)
                        nc.vector.tensor_scalar(
                            sqy[:], sqx[:], R2, None, Alu.is_le, Alu.add,
                            accum_out=pcnt[:, rt * nct + c: rt * nct + c + 1])

            dens_t = dpp.tile([dp, df], F32)
            if nct > 1:
                nc.vector.reduce_sum(
                    dens_t[:],
                    pcnt[:].rearrange("p (a b) -> p a b", a=df),
                    axis=mybir.AxisListType.X)
            else:
                nc.vector.tensor_copy(dens_t[:], pcnt[:])

            # relayout [128, df] (j = rt*128 + p) -> linear dram
            dd2 = dens_dram[:].rearrange("(a b) -> a b", a=df)  # [df, 128]
            nc.sync.dma_start(dd2.transpose([1, 0]), dens_t[:])
            nc.sync.dma_start(dens_out[:], dens_dram[:])

        with tc.tile_pool(name="fps", bufs=1) as pp:
            xft = pp.tile([fp, ff], F32)
            yft = pp.tile([fp, ff], F32)
            z2ft = pp.tile([fp, ff], F32)
            iot = pp.tile([fp, ff], F32)
            pent = pp.tile([fp, ff], F32)
            penf = pp.tile([fp, ff], F32)
            key = pp.tile([fp, ff], F32)
            trace = pp.tile([fp, npoint + fps_unroll + 66], F32)

            nc.sync.dma_start(xft[:], xf_d[:])
            nc.sync.dma_start(yft[:], yf_d[:])
            nc.sync.dma_start(iot[:], iota_d[:])
            zf_tmp = pp.tile([fp, ff], F32)
            nc.sync.dma_start(zf_tmp[:], zf_d[:])
            nc.vector.tensor_scalar(z2ft[:], zf_tmp[:], 2.0, None, Alu.mult)

            # density in fps layout (j = p*ff + c) + reciprocal
            nc.sync.dma_start(penf[:], dens_dram[:].rearrange("(a b) -> a b",
                                                              a=fp))
            nc.vector.reciprocal(pent[:], penf[:])

            # ---------------- FPS init ----------------
            # key-state: round(MD_INIT * pen); see module docstring.
            nc.vector.tensor_scalar(key[:], pent[:], MD_INIT, None, Alu.mult)
            nc.vector.memset(trace[:], 0.0)

            seed_t = pp.tile([1, 4], F32)
            nc.sync.dma_start(seed_t[:], seed_d[:])

            ax = pp.tile([fp, ff], F32)
            ay_a = pp.tile([fp, ff], F32)
            ay_b = pp.tile([fp, ff], F32)
            az_a = pp.tile([fp, ff], F32)
            az_b = pp.tile([fp, ff], F32)
            s12 = pp.tile([fp, ff], F32)
            dd = pp.tile([fp, ff], F32)
            dp_t = pp.tile([fp, ff], F32)
            junk4 = pp.tile([fp, 4, ff], F32)
            rowmax = pp.tile([fp, 1], F32)
            parmax = pp.tile([fp, 1], F32)
            gcs = pp.tile([fp, 4], F32)   # per-partition (idx, px, py, pz2)
            pm = pp.tile([fp, 1], F32)    # winner-partition mask
            gfin = pp.tile([fp, 4], F32)  # gated candidates
            gc = pp.tile([fp, 4], F32)    # globally reduced + broadcast

            # seed: gc = (0, x0, y0, 2*z0) on every partition
            nc.gpsimd.partition_broadcast(gc[:], seed_t[:])

            def body(iv):
                # |dx| on ACT: Abs(-x + px); |2dz| via max pair on DVE;
                # |dy| on ACT or DVE per `ay_act`
                if not ay_act:
                    nc.vector.tensor_scalar(ay_b[:], yft[:], -1.0,
                                            gc[:, 2:3], Alu.mult, Alu.add)
                    nc.vector.tensor_scalar(ay_a[:], yft[:], gc[:, 2:3],
                                            None, Alu.subtract)
                nc.vector.tensor_scalar(az_b[:], z2ft[:], -1.0, gc[:, 3:4],
                                        Alu.mult, Alu.add)
                nc.vector.tensor_scalar(az_a[:], z2ft[:], gc[:, 3:4], None,
                                        Alu.subtract)
                nc.scalar.activation(ax[:], xft[:], Act.Abs,
                                     bias=gc[:, 1:2], scale=-1.0)
                if ay_act:
                    nc.scalar.activation(ay_a[:], yft[:], Act.Abs,
                                         bias=gc[:, 2:3], scale=-1.0)
                else:
                    nc.vector.tensor_tensor(ay_a[:], ay_a[:], ay_b[:],
                                            Alu.max)
                nc.vector.tensor_tensor(az_a[:], az_a[:], az_b[:], Alu.max)
                nc.vector.tensor_tensor(s12[:], ax[:], ay_a[:], Alu.add)
                nc.vector.tensor_tensor(dd[:], s12[:], az_a[:], Alu.add)
                nc.vector.tensor_tensor(dp_t[:], dd[:], pent[:], Alu.mult)
                # key = min(key, dp); rowmax = max over free of new key
                nc.vector.tensor_tensor(key[:], key[:], dp_t[:], Alu.min)
                nc.vector.reduce_max(rowmax[:], key[:],
                                     axis=mybir.AxisListType.X)
                # global max (folded + broadcast) on GPSIMD, overlapped with
                # the per-partition candidate extraction below
                nc.gpsimd.partition_all_reduce(
                    parmax[:], rowmax[:], 128, bass_isa.ReduceOp.max)
                # per-partition candidate: (key >= rowmax_p) * v, sum of free
                # (separate junk output slices keep the 4 stts pipelined)
                nc.vector.scalar_tensor_tensor(
                    junk4[:, 0, :], key[:], rowmax[:, 0:1], iot[:],
                    op0=Alu.is_ge, op1=Alu.mult, accum_out=gcs[:, 0:1])
                nc.vector.scalar_tensor_tensor(
                    junk4[:, 1, :], key[:], rowmax[:, 0:1], xft[:],
                    op0=Alu.is_ge, op1=Alu.mult, accum_out=gcs[:, 1:2])
                nc.vector.scalar_tensor_tensor(
                    junk4[:, 2, :], key[:], rowmax[:, 0:1], yft[:],
                    op0=Alu.is_ge, op1=Alu.mult, accum_out=gcs[:, 2:3])
                nc.vector.scalar_tensor_tensor(
                    junk4[:, 3, :], key[:], rowmax[:, 0:1], z2ft[:],
                    op0=Alu.is_ge, op1=Alu.mult, accum_out=gcs[:, 3:4])
                # gate: winner partition keeps its candidate, others zero
                # (on GPSIMD: engine-local with the all-reduces, no extra
                # cross-engine semaphore hops)
                nc.gpsimd.tensor_scalar(pm[:], rowmax[:], parmax[:, 0:1],
                                        None, Alu.is_ge)
                nc.gpsimd.tensor_scalar(gfin[:], gcs[:], pm[:, 0:1], None,
                                        Alu.mult)
                nc.gpsimd.partition_all_reduce(
                    gc[:], gfin[:], 128, bass_isa.ReduceOp.add)
                # record selected index (output position iv+1)
                nc.vector.tensor_copy(
                    trace[0:1, 1:][:, bass.DynSlice(iv, 1)], gc[0:1, 0:1])

            if loop_mode == "unrolled":
                for t in range(npoint):
                    body(t)
            else:
                for _rep in range(bench_repeats):
                    with tc.For_i(0, npoint, fps_unroll) as iv:
                        for k in range(fps_unroll):
                            body(iv + k)

            # ---------------- outputs ----------------
            idx32 = pp.tile([1, npoint], I32)
            nc.vector.tensor_copy(idx32[:], trace[0:1, 0:npoint])
            nc.sync.dma_start(idx_out[:].rearrange("(a b) -> a b", a=1),
                              idx32[:])

    nc.finalize()
    return nc


def make_in_maps(points, n=N, n_cores=8, ntrip=NPOINT):
    """Per-core host-side input layouts. Core c handles batch c % B."""
    fp, ff = 128, n // 128
    dp, df = 128, n // 128
    iota = np.arange(n, dtype=np.float32).reshape(fp, ff)
    in_maps = []
    for c in range(n_cores):
        b = c % points.shape[0]
        p = np.ascontiguousarray(points[b])  # [n, 3] f32
        seed = np.array([[0.0, p[0, 0], p[0, 1],
                          np.float32(2.0) * p[0, 2]]], dtype=np.float32)
        m = {
            "xf": p[:, 0].reshape(fp, ff).copy(),
            "yf": p[:, 1].reshape(fp, ff).copy(),
            "zf": p[:, 2].reshape(fp, ff).copy(),
            "iota": iota,
            "xi": np.ascontiguousarray(p[:, 0].reshape(df, dp).T),
            "yi": np.ascontiguousarray(p[:, 1].reshape(df, dp).T),
            "zi": np.ascontiguousarray(p[:, 2].reshape(df, dp).T),
            "xj": p[:, 0].reshape(1, n).copy(),
            "yj": p[:, 1].reshape(1, n).copy(),
            "zj": p[:, 2].reshape(1, n).copy(),
            "seed": seed,
        }
        in_maps.append(m)
    return in_maps


_NC_CACHE = {}


def kernel(points, features=None, npoint=None, **_unused):
    """Full-input entry point: points [4, 8192, 3] f32 -> [4, 2048] int32."""
    global LAST_PERF
    points = np.asarray(points, dtype=np.float32)
    assert points.shape == (B, N, 3), points.shape
    npt = int(npoint) if npoint is not None else NPOINT
    assert npt == NPOINT, f"kernel hardcodes npoint={NPOINT}, got {npt}"

    if "nc" not in _NC_CACHE:
        _NC_CACHE["nc"] = build_nc()
    nc = _NC_CACHE["nc"]

    in_maps = make_in_maps(points)
    res = run_bass_kernel_spmd(nc, in_maps, core_ids=list(range(8)))
    LAST_PERF = res
    out = np.stack([res.results[b]["idx_out"] for b in range(B)], axis=0)
    return out.astype(np.int32)
